# revision 19
# baseline (speedup 1.0000x reference)
"""Bass/Tile kernel for chunkwise retention (nn_ChunkwiseRetention).

Algorithm (per core = one batch element, seq 4000, B=5, 800 chunks):
superchunks of G=25 chunks (125 positions). The host pre-scales
xqT columns by g6^j and xkT by g6^-j (j = global chunk index), which
folds the entire cross-chunk decay into the projections: the cross
mask becomes 0/1, the carry is Q~ @ U with no rescale, and the state
update needs no scaling at all.

Per superchunk s: Q~^T,K~^T (dim-major, projected 4 superchunks at a
time at N=500) and K~,V (pos-major) projections; P~^T = K~ @ Q~^T;
masked matmuls accumulate cross + intra (+5-row shift via
free-dim-shifted stationary) + seam (previous superchunk's tail
stationary x previous V) + carry (Q~ @ U) into one PSUM window;
running state U in one PSUM bank (zero-matmul init, per-element
has_written accumulation).

All matmul operands are bf16 (host casts the scaled inputs): 1
cycle/row on the PE at any moving size (f32r needs moving>=256, f32 is
4 cycles/row), so the P~^T matmuls (N=125) run 4x faster than f32.
bf16 keeps f32's exponent range, which the g6^+-j scaling needs.
PSUM accumulation stays f32; the final output is written f32.

Engine split (GPSIMD cannot touch PSUM, so evacuations go DVE/ACT):
DVE P~^T evac + kv evac + wall copy; ACT qt/kt (single 2-segment ops)
+ ut; Pool (GPSIMD) the fused mask-multiply (SBUF-only). K~ pos-major
comes from PE transposes of K~^T into the kv bank (saves 256 PE
rows/superchunk vs projecting K twice). Output DMAs are paired (two
superchunks per DMA) to halve SP DGE issue pressure.

PSUM banks (8): qkt(shared) 2 + kv 2 + pt 1 + wt 2 + u 1.
"""
import numpy as np
import ml_dtypes

import concourse.bass as bass
import concourse.mybir as mybir
import concourse.tile as tile

GAMMA = 0.9865
B = 5
SEQ = 4000
FEAT = 256
DIM = 256
G = 25
GP = G * B            # 125
NSC = SEQ // GP       # 32
LG = 4                # superchunks per projection/load group
LGP = LG * GP         # 500
F32 = mybir.dt.float32
F32R = mybir.dt.float32r
BF16 = mybir.dt.bfloat16
g6 = float(np.float64(GAMMA) ** 6)
COPY = mybir.ActivationFunctionType.Copy

# const blob column layout (f32 masks for the DVE multiplies). wct and wit
# sit 130 columns apart so ONE fused DVE tensor_mul (2-segment AP, outer
# stride 130) produces both masked stationaries from a double-read of P~^T.
C_WCT = 0            # [0:125)   0/1 strict lower-block-triangular cross mask
C_WIT = 130          # [130:255) intra decay mask (rows 0:125)
C_Z = 255            # [255:767) zeros (row 0 used as zero matmul operand)
C_END = 767


def make_const_blob():
    t = np.arange(GP) // B
    p = np.arange(GP) % B
    tb, ta = t[:, None], t[None, :]
    wct01 = (tb < ta).astype(np.float32)
    qb, pa = p[:, None], p[None, :]
    wit = np.where((tb == ta) & (pa >= qb),
                   np.float64(GAMMA) ** (qb - pa), 0.0).astype(np.float32)
    blob = np.zeros((128, C_END), np.float32)
    blob[0:GP, C_WCT:C_WCT + 125] = wct01
    blob[0:GP, C_WIT:C_WIT + 125] = wit
    return blob


def build_kernel(nc: bass.Bass):
    xqT = nc.dram_tensor("xqT", [FEAT, SEQ], BF16, kind="ExternalInput").ap()
    xkT = nc.dram_tensor("xkT", [FEAT, SEQ], BF16, kind="ExternalInput").ap()
    xvT = nc.dram_tensor("xvT", [FEAT, SEQ], BF16, kind="ExternalInput").ap()
    wqkv = nc.dram_tensor("wqkv", [FEAT, 3 * DIM], BF16, kind="ExternalInput").ap()
    out = nc.dram_tensor("out", [SEQ, DIM], F32, kind="ExternalOutput").ap()

    blob_np = make_const_blob()
    mm = nc.tensor.matmul

    with tile.TileContext(nc) as tc:
        with (
            tc.tile_pool(name="consts", bufs=1) as cpool,
            tc.tile_pool(name="xin", bufs=2) as xpool,
            tc.tile_pool(name="work", bufs=2) as spool,
            tc.tile_pool(name="psT", bufs=1, space="PSUM") as psT,
            tc.tile_pool(name="psP", bufs=2, space="PSUM") as psP,
            tc.tile_pool(name="psPT", bufs=1, space="PSUM") as psPT,
            tc.tile_pool(name="psW", bufs=2, space="PSUM") as psW,
            tc.tile_pool(name="psU", bufs=1, space="PSUM") as psU,
        ):
            # --- constants to SBUF (blob via SP, weights+identity via ACT's
            # DGE so the startup DMAs issue in parallel) ---
            blob_sb = cpool.tile([128, C_END], F32, name="blob_sb")
            nc.sync.dma_start(out=blob_sb,
                              in_=nc.inline_tensor(blob_np, "cblob").ap())
            idn_np = np.eye(128, dtype=np.float32).astype(ml_dtypes.bfloat16)
            idn_sb = cpool.tile([128, 128], BF16, name="idn_sb")
            nc.scalar.dma_start(out=idn_sb,
                                in_=nc.inline_tensor(idn_np, "idn").ap())
            w_sb = cpool.tile([128, 2, 3 * DIM], BF16, name="w_sb")
            nc.scalar.dma_start(out=w_sb,
                                in_=wqkv.rearrange("(h p) d -> p h d", p=128))
            wv_sb = w_sb[:, :, 512:768]

            u_ps = psU.tile([128, 512], F32, name="u_state")

            # preamble: absorb the weights-DMA wait into one dummy matmul and
            # the const-blob DMA wait into one dummy DVE copy (fewer split
            # waits); zero-matmul initializes the U bank's data + has_written
            # bits so the per-superchunk state matmuls can all accumulate.
            nc.tensor.matmul(u_ps[0:1, 0:1], w_sb[:, 0, 0:1], w_sb[:, 0, 0:1],
                             start=True, stop=True, skip_group_check=True)
            scratch_sb = spool.tile([1, 1], F32, name="scratch", tag="scratch")
            nc.vector.tensor_copy(scratch_sb, blob_sb[0:1, 0:1])
            nc.tensor.matmul(u_ps,
                             blob_sb[0:1, C_Z:C_Z + 128].bitcast(F32R),
                             blob_sb[0:1, C_Z:C_Z + 512].bitcast(F32R),
                             start=True, stop=True, skip_group_check=True)

            # persistent combined mask stationaries (manual triple-buffer):
            # cols 0:125 = mpc (cross, rewritten fully each superchunk);
            # cols 125:375 = mpi region (shift trick: write 130:255, main
            # read 125:250, tail read 250:375); zeros memset once.
            # One fused DVE op writes both segments (outer stride 130).
            mp_bufs = []
            for i_ in range(3):
                mb_ = spool.tile([125, 375], BF16, name=f"mp_{i_}", tag=f"mp_{i_}",
                                 bufs=1)
                nc.vector.memset(mb_[:, 125:130], 0.0)
                nc.vector.memset(mb_[:, 255:375], 0.0)
                mp_bufs.append(mb_)

            prev_mp = prev_v = None
            xg = {}
            qkt_sb = {}
            wpair = {}

            def load_group(gidx):
                gsl = slice(gidx * LGP, (gidx + 1) * LGP)
                xq_g = xpool.tile([128, 2, LGP], BF16, name=f"xq_{gidx}", tag="xq")
                xk_g = xpool.tile([128, 2, LGP], BF16, name=f"xk_{gidx}", tag="xk")
                xv_g = xpool.tile([128, 2, LGP], BF16, name=f"xv_{gidx}", tag="xv")
                # group 0 spreads across both DGE queues to cut startup latency
                eng_q = nc.scalar if gidx == 0 else nc.sync
                eng_q.dma_start(out=xq_g, in_=xqT[:, gsl].rearrange("(h p) a -> p h a", p=128))
                nc.sync.dma_start(out=xk_g, in_=xkT[:, gsl].rearrange("(h p) a -> p h a", p=128))
                nc.sync.dma_start(out=xv_g, in_=xvT[:, gsl].rearrange("(h p) a -> p h a", p=128))
                xg["x"] = (xq_g, xk_g, xv_g)
                # Q~^T / K~^T projections for the group (N=500), via a shared
                # psum tag: d-lo cols 0:500 (bank 0), d-hi cols 512:1012
                # (bank 1), each bank one closed accumulation group
                qt_sb = spool.tile([128, 1000], BF16, name=f"qt_{gidx}", tag="qt")
                kt_sb = spool.tile([128, 1000], BF16, name=f"kt_{gidx}", tag="kt")
                qkt_q = psT.tile([128, 1024], F32, name=f"qkt_q_{gidx}", tag="qkt")
                for off, dlo in ((0, 0), (512, 128)):
                    for h in (0, 1):
                        mm(qkt_q[:, off:off + LGP], w_sb[:, h, dlo:dlo + 128],
                           xq_g[:, h, :], start=(h == 0), stop=(h == 1))
                nc.scalar.activation(
                    qt_sb.rearrange("p (a b) -> p a b", b=500),
                    qkt_q.rearrange("p (a b) -> p a b", b=512)[:, :, 0:500], COPY)
                qkt_k = psT.tile([128, 1024], F32, name=f"qkt_k_{gidx}", tag="qkt")
                for off, dlo in ((0, 256), (512, 384)):
                    for h in (0, 1):
                        mm(qkt_k[:, off:off + LGP], w_sb[:, h, dlo:dlo + 128],
                           xk_g[:, h, :], start=(h == 0), stop=(h == 1))
                nc.scalar.activation(
                    kt_sb.rearrange("p (a b) -> p a b", b=500),
                    qkt_k.rearrange("p (a b) -> p a b", b=512)[:, :, 0:500], COPY)
                qkt_sb["qk"] = (qt_sb, kt_sb)

            def prep_sc(s):
                """K~/V + P~^T + masked stationaries for superchunk s."""
                gidx, ls = divmod(s, LG)
                if ls == 0:
                    load_group(gidx)
                xq_g, xk_g, xv_g = xg["x"]
                qt_sb, kt_sb = qkt_sb["qk"]
                lsl = slice(ls * GP, (ls + 1) * GP)
                xv_s = xv_g[:, :, lsl]
                qlo = qt_sb[:, ls * GP:(ls + 1) * GP]
                qhi = qt_sb[:, 500 + ls * GP:500 + (ls + 1) * GP]
                klo = kt_sb[:, ls * GP:(ls + 1) * GP]
                khi = kt_sb[:, 500 + ls * GP:500 + (ls + 1) * GP]

                # K~ pos-major via PE transpose of K~^T (bf16, written into
                # the f32 kv bank's first 512B via bitcast views); V pos-major
                # projected from x (f32 cols 256:512)
                kv = psP.tile([125, 512], F32, name=f"kv_{s}", tag="kv")
                nc.tensor.transpose(kv[:, 0:64].bitcast(BF16), klo, idn_sb)
                nc.tensor.transpose(kv[:, 64:128].bitcast(BF16), khi, idn_sb)
                for h in (0, 1):
                    mm(kv[:, 256:512], xv_s[:, h, :], wv_sb[:, h, :],
                       start=(h == 0), stop=(h == 1))
                kv_sb = spool.tile([125, 512], BF16, name=f"kv_sb_{s}", tag="kvsb", bufs=3)
                nc.vector.tensor_copy(kv_sb[:, 0:256], kv[:, 0:128].bitcast(BF16))
                nc.vector.tensor_copy(kv_sb[:, 256:512], kv[:, 256:512])
                k_sb = kv_sb[:, 0:256]
                v_sb = kv_sb[:, 256:512]

                # P~^T = K~ @ Q~^T (bf16: 1 cycle/row at any N)
                pt_ps = psPT.tile([125, 125], F32, name=f"pt_{s}", tag="pt")
                mm(pt_ps, klo, qlo, start=True, stop=False)
                mm(pt_ps, khi, qhi, start=False, stop=True)
                pt_sb = spool.tile([125, 125], F32, name=f"pt_sb_{s}", tag="ptsb",
                                   bufs=2)
                nc.vector.tensor_copy(pt_sb, pt_ps)

                # one fused GPSIMD op: {mpc (cols 0:125), shifted mpi
                # (130:255)} = P~^T (read twice, stride-0) * {wct, wit}
                mp_sb = mp_bufs[s % 3]
                mp_out = mp_sb[:, 0:260].rearrange(
                    "p (a b) -> p a b", b=130)[:, :, 0:125]
                masks_in = blob_sb[0:GP, C_WCT:C_WCT + 260].rearrange(
                    "p (a b) -> p a b", b=130)[:, :, 0:125]
                pt_in = pt_sb.unsqueeze(1).broadcast_to((GP, 2, 125))
                nc.gpsimd.tensor_mul(mp_out, pt_in, masks_in)
                return dict(k_sb=k_sb, v_sb=v_sb, mp_sb=mp_sb, qlo=qlo, qhi=qhi)

            st = prep_sc(0)
            for s in range(NSC):
                k_sb, v_sb = st["k_sb"], st["v_sb"]
                mp_sb = st["mp_sb"]
                qlo, qhi = st["qlo"], st["qhi"]

                # --- window accumulation (one closed group). The seam
                # (intra tail of chunk s*G-1) is added directly from the
                # previous superchunk's tail stationary and V: rows 5:125 of
                # that matmul multiply zero columns and accumulate zeros. ---
                wt = psW.tile([125, 256], F32, name=f"wt_{s}", tag="wt")
                mm(wt, mp_sb[:, 0:125], v_sb, start=True, stop=False)
                mm(wt, mp_sb[:, 125:250], v_sb, start=False, stop=(s == 0))
                if s > 0:
                    ut_sb = spool.tile([128, 512], BF16, name=f"ut_{s}", tag="ut")
                    nc.scalar.activation(ut_sb, u_ps, COPY)
                    mm(wt, prev_mp[:, 250:375], prev_v, start=False, stop=False)
                    mm(wt, qlo, ut_sb[:, 0:256], start=False, stop=False)
                    mm(wt, qhi, ut_sb[:, 256:512], start=False, stop=True)

                # --- state update (accumulates; U bank bits set by zero-mm) ---
                mm(u_ps[:, 0:256], k_sb[:, 0:128], v_sb,
                   start=False, stop=True, skip_group_check=True)
                mm(u_ps[:, 256:512], k_sb[:, 128:256], v_sb,
                   start=False, stop=True, skip_group_check=True)

                # pipeline: prepare s+1 so PE has projection/PT work in flight
                # while DVE produces the next masks
                if s + 1 < NSC:
                    st = prep_sc(s + 1)

                # --- output: DVE evac; superchunks 1..30 pair up (two windows
                # per [125,512] tile, one DMA per pair) to halve DGE issues ---
                if s == 0:
                    wall_sb = spool.tile([125, 256], F32, name="wall_0",
                                         tag="wall0")
                    nc.vector.tensor_copy(wall_sb, wt)
                    nc.sync.dma_start(out=out[0:GP - B], in_=wall_sb[B:GP])
                elif s == NSC - 1:
                    wall_sb = spool.tile([125, 256], F32, name=f"wall_{s}",
                                         tag="wallz")
                    nc.vector.tensor_copy(wall_sb, wt)
                    nc.sync.dma_start(out=out[s * GP - B: s * GP - B + GP],
                                      in_=wall_sb)
                else:
                    half = (s - 1) % 2
                    if half == 0:
                        wpair["t"] = spool.tile([125, 512], F32,
                                                name=f"wallp_{s}", tag="wallp",
                                                bufs=2)
                    wp = wpair["t"]
                    nc.vector.tensor_copy(wp[:, half * 256:half * 256 + 256], wt)
                    if half == 1:
                        base = (s - 1) * GP - B
                        nc.sync.dma_start(
                            out=out[base: base + 2 * GP].rearrange(
                                "(a p) d -> p a d", a=2),
                            in_=wp)
                prev_mp, prev_v = mp_sb, v_sb

            # final output chunk 799 = intra tail of the last superchunk
            wtf = psW.tile([125, 256], F32, name="wt_final", tag="wt")
            mm(wtf, prev_mp[:, 250:375], prev_v, start=True, stop=True)
            wallf_sb = spool.tile([5, 256], F32, name="wallf", tag="wallf")
            nc.vector.tensor_copy(wallf_sb, wtf[0:5])
            nc.sync.dma_start(out=out[SEQ - B:SEQ], in_=wallf_sb)

    return nc


def _col_scales():
    j = np.arange(SEQ) // B          # global chunk index
    sq = (np.float64(g6) ** j).astype(np.float32)
    sk = (np.float64(g6) ** (-j)).astype(np.float32)
    return sq, sk


def prep_core_inputs(xq2d, xk2d, xv2d, wqkv):
    sq, sk = _col_scales()
    bf = ml_dtypes.bfloat16
    return {
        "xqT": np.ascontiguousarray((xq2d.T * sq[None, :]).astype(bf)),
        "xkT": np.ascontiguousarray((xk2d.T * sk[None, :]).astype(bf)),
        "xvT": np.ascontiguousarray(xv2d.T.astype(bf)),
        "wqkv": wqkv,
    }


def make_in_maps(inputs):
    """inputs: dict from setup_inputs (full batch). Returns per-core in_maps."""
    xq, xk, xv = inputs["xq"], inputs["xk"], inputs["xv"]
    wqkv = np.ascontiguousarray(np.concatenate(
        [np.asarray(inputs["Wq"], dtype=np.float32),
         np.asarray(inputs["Wk"], dtype=np.float32),
         np.asarray(inputs["Wv"], dtype=np.float32)],
        axis=1).astype(ml_dtypes.bfloat16))
    in_maps = []
    for b in range(8):
        in_maps.append(prep_core_inputs(
            np.asarray(xq[b], dtype=np.float32),
            np.asarray(xk[b], dtype=np.float32),
            np.asarray(xv[b], dtype=np.float32), wqkv))
    return in_maps


_NC_CACHE = {}


def _get_nc():
    if "nc" not in _NC_CACHE:
        from concourse import bacc
        nc = bacc.Bacc("TRN2", target_bir_lowering=False, debug=False)
        build_kernel(nc)
        nc.compile()
        _NC_CACHE["nc"] = nc
    return _NC_CACHE["nc"]


def run(inputs, trace=False, **kwargs):
    """Run on 8 NeuronCores; returns (output [8,4000,256], BassKernelResults)."""
    from concourse.bass_utils import run_bass_kernel_spmd

    nc = _get_nc()
    in_maps = make_in_maps(inputs)
    res = run_bass_kernel_spmd(nc, in_maps, core_ids=list(range(8)),
                               trace=trace, **kwargs)
    out = np.stack([r["out"] for r in res.results], axis=0)
    return out, res


def kernel(**inputs) -> np.ndarray:
    out, _ = run(inputs)
    return out


# revision 22
# speedup vs baseline: 1.1510x; 1.1510x over previous
"""Bass/Tile kernel for chunkwise retention (nn_ChunkwiseRetention).

Algorithm (per core = one batch element, seq 4000, B=5, 800 chunks):
superchunks of G=25 chunks (125 positions). The host pre-scales
xqT columns by g6^j and xkT by g6^-j (j = global chunk index), which
folds the entire cross-chunk decay into the projections: the cross
mask becomes 0/1, the carry is Q~ @ U with no rescale, and the state
update needs no scaling at all.

Per superchunk s: Q~^T,K~^T (dim-major, projected 4 superchunks at a
time at N=500) and K~,V (pos-major) projections; P~^T = K~ @ Q~^T;
masked matmuls accumulate cross + intra (+5-row shift via
free-dim-shifted stationary) + seam (previous superchunk's tail
stationary x previous V) + carry (Q~ @ U) into one PSUM window;
running state U in one PSUM bank (zero-matmul init, per-element
has_written accumulation).

All matmul operands are bf16 (host casts the scaled inputs): 1
cycle/row on the PE at any moving size (f32r needs moving>=256, f32 is
4 cycles/row), so the P~^T matmuls (N=125) run 4x faster than f32.
bf16 keeps f32's exponent range, which the g6^+-j scaling needs.
PSUM accumulation stays f32; the final output is written f32.

Engine split (GPSIMD cannot touch PSUM, so evacuations go DVE/ACT):
DVE P~^T evac + kv evac + wall copy; ACT qt/kt (single 2-segment ops)
+ ut; Pool (GPSIMD) the fused mask-multiply (SBUF-only). K~ pos-major
comes from PE transposes of K~^T into the kv bank (saves 256 PE
rows/superchunk vs projecting K twice). Output DMAs are paired (two
superchunks per DMA) to halve SP DGE issue pressure.

PSUM banks (8): qkt(shared) 2 + kv 2 + pt 1 + wt 2 + u 1.
"""
import numpy as np
import ml_dtypes

import concourse.bass as bass
import concourse.mybir as mybir
import concourse.tile as tile

GAMMA = 0.9865
B = 5
SEQ = 4000
FEAT = 256
DIM = 256
G = 25
GP = G * B            # 125
NSC = SEQ // GP       # 32
LG = 4                # superchunks per projection/load group
LGP = LG * GP         # 500
F32 = mybir.dt.float32
F32R = mybir.dt.float32r
BF16 = mybir.dt.bfloat16
g6 = float(np.float64(GAMMA) ** 6)
COPY = mybir.ActivationFunctionType.Copy

# const blob column layout (f32 masks for the DVE multiplies). wct and wit
# sit 130 columns apart so ONE fused DVE tensor_mul (2-segment AP, outer
# stride 130) produces both masked stationaries from a double-read of P~^T.
C_WCT = 0            # [0:125)   0/1 strict lower-block-triangular cross mask
C_WIT = 130          # [130:255) intra decay mask (rows 0:125)
C_Z = 255            # [255:767) zeros (row 0 used as zero matmul operand)
C_END = 767


def make_const_blob():
    t = np.arange(GP) // B
    p = np.arange(GP) % B
    tb, ta = t[:, None], t[None, :]
    wct01 = (tb < ta).astype(np.float32)
    qb, pa = p[:, None], p[None, :]
    wit = np.where((tb == ta) & (pa >= qb),
                   np.float64(GAMMA) ** (qb - pa), 0.0).astype(np.float32)
    blob = np.zeros((128, C_END), np.float32)
    blob[0:GP, C_WCT:C_WCT + 125] = wct01
    blob[0:GP, C_WIT:C_WIT + 125] = wit
    return blob


def build_kernel(nc: bass.Bass):
    xqT = nc.dram_tensor("xqT", [FEAT, SEQ], BF16, kind="ExternalInput").ap()
    xkT = nc.dram_tensor("xkT", [FEAT, SEQ], BF16, kind="ExternalInput").ap()
    xvT = nc.dram_tensor("xvT", [FEAT, SEQ], BF16, kind="ExternalInput").ap()
    wqkv = nc.dram_tensor("wqkv", [FEAT, 3 * DIM], BF16, kind="ExternalInput").ap()
    out = nc.dram_tensor("out", [SEQ, DIM], F32, kind="ExternalOutput").ap()

    blob_np = make_const_blob()
    mm = nc.tensor.matmul

    with tile.TileContext(nc) as tc:
        with (
            tc.tile_pool(name="consts", bufs=1) as cpool,
            tc.tile_pool(name="xin", bufs=2) as xpool,
            tc.tile_pool(name="work", bufs=2) as spool,
            tc.tile_pool(name="psT", bufs=1, space="PSUM") as psT,
            tc.tile_pool(name="psP", bufs=2, space="PSUM") as psP,
            tc.tile_pool(name="psPT", bufs=1, space="PSUM") as psPT,
            tc.tile_pool(name="psW", bufs=2, space="PSUM") as psW,
            tc.tile_pool(name="psU", bufs=1, space="PSUM") as psU,
        ):
            # --- constants to SBUF (weights first on ACT's DGE queue — the
            # preamble and first projections wait on them; blob/identity on
            # SP so the startup DMAs issue in parallel) ---
            w_sb = cpool.tile([128, 2, 3 * DIM], BF16, name="w_sb")
            nc.scalar.dma_start(out=w_sb,
                                in_=wqkv.rearrange("(h p) d -> p h d", p=128))
            blob_sb = cpool.tile([128, C_END], F32, name="blob_sb")
            nc.sync.dma_start(out=blob_sb,
                              in_=nc.inline_tensor(blob_np, "cblob").ap())
            idn_np = np.eye(128, dtype=np.float32).astype(ml_dtypes.bfloat16)
            idn_sb = cpool.tile([128, 128], BF16, name="idn_sb")
            nc.sync.dma_start(out=idn_sb,
                              in_=nc.inline_tensor(idn_np, "idn").ap())
            wv_sb = w_sb[:, :, 512:768]

            u_ps = psU.tile([128, 512], F32, name="u_state")

            # preamble: absorb the weights-DMA wait into one dummy matmul and
            # the const-blob DMA wait into one dummy DVE copy (fewer split
            # waits); zero-matmul initializes the U bank's data + has_written
            # bits so the per-superchunk state matmuls can all accumulate.
            nc.tensor.matmul(u_ps[0:1, 0:1], w_sb[:, 0, 0:1], w_sb[:, 0, 0:1],
                             start=True, stop=True, skip_group_check=True)
            scratch_sb = spool.tile([1, 1], F32, name="scratch", tag="scratch")
            nc.vector.tensor_copy(scratch_sb, blob_sb[0:1, 0:1])
            nc.tensor.matmul(u_ps,
                             blob_sb[0:1, C_Z:C_Z + 128].bitcast(F32R),
                             blob_sb[0:1, C_Z:C_Z + 512].bitcast(F32R),
                             start=True, stop=True, skip_group_check=True)

            # persistent combined mask stationaries (manual triple-buffer):
            # cols 0:125 = mpc (cross, rewritten fully each superchunk);
            # cols 125:375 = mpi region (shift trick: write 130:255, main
            # read 125:250, tail read 250:375); zeros memset once.
            # One fused DVE op writes both segments (outer stride 130).
            mp_bufs = []
            for i_ in range(3):
                mb_ = spool.tile([125, 375], BF16, name=f"mp_{i_}", tag=f"mp_{i_}",
                                 bufs=1)
                nc.vector.memset(mb_[:, 125:130], 0.0)
                nc.vector.memset(mb_[:, 255:375], 0.0)
                mp_bufs.append(mb_)

            prev_mp = prev_v = None
            xg = {}
            qkt_sb = {}
            wpair = {}

            def load_x(gidx):
                gsl = slice(gidx * LGP, (gidx + 1) * LGP)
                xq_g = xpool.tile([128, 2, LGP], BF16, name=f"xq_{gidx}", tag="xq")
                xk_g = xpool.tile([128, 2, LGP], BF16, name=f"xk_{gidx}", tag="xk")
                xv_g = xpool.tile([128, 2, LGP], BF16, name=f"xv_{gidx}", tag="xv")
                # group 0 spreads across both DGE queues to cut startup latency
                eng_q = nc.scalar if gidx == 0 else nc.sync
                nc.sync.dma_start(out=xq_g, in_=xqT[:, gsl].rearrange("(h p) a -> p h a", p=128))
                eng_q.dma_start(out=xk_g, in_=xkT[:, gsl].rearrange("(h p) a -> p h a", p=128))
                nc.sync.dma_start(out=xv_g, in_=xvT[:, gsl].rearrange("(h p) a -> p h a", p=128))
                xg[gidx] = (xq_g, xk_g, xv_g)

            def project_group(gidx):
                """Q~^T / K~^T projections for the group (N=500), via a shared
                psum tag: d-lo cols 0:500 (bank 0), d-hi cols 512:1012
                (bank 1), each bank one closed accumulation group. K first so
                its evac (needed by the boundary transpose + P~^T) overlaps
                the Q projection."""
                xq_g, xk_g, xv_g = xg[gidx]
                qt_sb = spool.tile([128, 1000], BF16, name=f"qt_{gidx}", tag="qt")
                kt_sb = spool.tile([128, 1000], BF16, name=f"kt_{gidx}", tag="kt")
                qkt_k = psT.tile([128, 1024], F32, name=f"qkt_k_{gidx}", tag="qkt")
                for off, dlo in ((0, 256), (512, 384)):
                    for h in (0, 1):
                        mm(qkt_k[:, off:off + LGP], w_sb[:, h, dlo:dlo + 128],
                           xk_g[:, h, :], start=(h == 0), stop=(h == 1))
                nc.scalar.activation(
                    kt_sb.rearrange("p (a b) -> p a b", b=500),
                    qkt_k.rearrange("p (a b) -> p a b", b=512)[:, :, 0:500], COPY)
                qkt_q = psT.tile([128, 1024], F32, name=f"qkt_q_{gidx}", tag="qkt")
                for off, dlo in ((0, 0), (512, 128)):
                    for h in (0, 1):
                        mm(qkt_q[:, off:off + LGP], w_sb[:, h, dlo:dlo + 128],
                           xq_g[:, h, :], start=(h == 0), stop=(h == 1))
                nc.scalar.activation(
                    qt_sb.rearrange("p (a b) -> p a b", b=500),
                    qkt_q.rearrange("p (a b) -> p a b", b=512)[:, :, 0:500], COPY)
                qkt_sb[gidx] = (qt_sb, kt_sb)

            def prep_sc(s):
                """K~/V + P~^T + masked stationaries for superchunk s.
                Group g+1's x-load fires at ls==1 and its projections at
                ls==2, two superchunks before the boundary, so the boundary
                never waits on the qt/kt evacuations."""
                gidx, ls = divmod(s, LG)
                if ls == 1 and gidx + 1 < NSC // LG:
                    load_x(gidx + 1)
                if ls == 2 and gidx + 1 < NSC // LG:
                    project_group(gidx + 1)
                    xg.pop(gidx - 1, None)
                    qkt_sb.pop(gidx - 1, None)
                xq_g, xk_g, xv_g = xg[gidx]
                qt_sb, kt_sb = qkt_sb[gidx]
                lsl = slice(ls * GP, (ls + 1) * GP)
                xv_s = xv_g[:, :, lsl]
                qlo = qt_sb[:, ls * GP:(ls + 1) * GP]
                qhi = qt_sb[:, 500 + ls * GP:500 + (ls + 1) * GP]
                klo = kt_sb[:, ls * GP:(ls + 1) * GP]
                khi = kt_sb[:, 500 + ls * GP:500 + (ls + 1) * GP]

                # K~ pos-major via PE transpose of K~^T (bf16, written into
                # the f32 kv bank's first 512B via bitcast views); V pos-major
                # projected from x (f32 cols 256:512)
                kv = psP.tile([125, 512], F32, name=f"kv_{s}", tag="kv")
                nc.tensor.transpose(kv[:, 0:64].bitcast(BF16), klo, idn_sb)
                nc.tensor.transpose(kv[:, 64:128].bitcast(BF16), khi, idn_sb)
                for h in (0, 1):
                    mm(kv[:, 256:512], xv_s[:, h, :], wv_sb[:, h, :],
                       start=(h == 0), stop=(h == 1))
                kv_sb = spool.tile([125, 512], BF16, name=f"kv_sb_{s}", tag="kvsb", bufs=3)
                nc.vector.tensor_copy(kv_sb[:, 0:256], kv[:, 0:128].bitcast(BF16))
                nc.vector.tensor_copy(kv_sb[:, 256:512], kv[:, 256:512])
                k_sb = kv_sb[:, 0:256]
                v_sb = kv_sb[:, 256:512]

                # P~^T = K~ @ Q~^T (bf16: 1 cycle/row at any N)
                pt_ps = psPT.tile([125, 125], F32, name=f"pt_{s}", tag="pt")
                mm(pt_ps, klo, qlo, start=True, stop=False)
                mm(pt_ps, khi, qhi, start=False, stop=True)
                pt_sb = spool.tile([125, 125], F32, name=f"pt_sb_{s}", tag="ptsb",
                                   bufs=2)
                nc.vector.tensor_copy(pt_sb, pt_ps)

                # one fused GPSIMD op: {mpc (cols 0:125), shifted mpi
                # (130:255)} = P~^T (read twice, stride-0) * {wct, wit}
                mp_sb = mp_bufs[s % 3]
                mp_out = mp_sb[:, 0:260].rearrange(
                    "p (a b) -> p a b", b=130)[:, :, 0:125]
                masks_in = blob_sb[0:GP, C_WCT:C_WCT + 260].rearrange(
                    "p (a b) -> p a b", b=130)[:, :, 0:125]
                pt_in = pt_sb.unsqueeze(1).broadcast_to((GP, 2, 125))
                nc.gpsimd.tensor_mul(mp_out, pt_in, masks_in)
                return dict(k_sb=k_sb, v_sb=v_sb, mp_sb=mp_sb, qlo=qlo, qhi=qhi)

            load_x(0)
            project_group(0)
            st = prep_sc(0)
            for s in range(NSC):
                k_sb, v_sb = st["k_sb"], st["v_sb"]
                mp_sb = st["mp_sb"]
                qlo, qhi = st["qlo"], st["qhi"]

                # --- window accumulation (one closed group). The seam
                # (intra tail of chunk s*G-1) is added directly from the
                # previous superchunk's tail stationary and V: rows 5:125 of
                # that matmul multiply zero columns and accumulate zeros. ---
                wt = psW.tile([125, 256], F32, name=f"wt_{s}", tag="wt")
                mm(wt, mp_sb[:, 0:125], v_sb, start=True, stop=False)
                mm(wt, mp_sb[:, 125:250], v_sb, start=False, stop=(s == 0))
                if s > 0:
                    ut_sb = spool.tile([128, 512], BF16, name=f"ut_{s}", tag="ut")
                    nc.scalar.activation(ut_sb, u_ps, COPY)
                    mm(wt, prev_mp[:, 250:375], prev_v, start=False, stop=False)
                    mm(wt, qlo, ut_sb[:, 0:256], start=False, stop=False)
                    mm(wt, qhi, ut_sb[:, 256:512], start=False, stop=True)

                # --- state update (accumulates; U bank bits set by zero-mm) ---
                mm(u_ps[:, 0:256], k_sb[:, 0:128], v_sb,
                   start=False, stop=True, skip_group_check=True)
                mm(u_ps[:, 256:512], k_sb[:, 128:256], v_sb,
                   start=False, stop=True, skip_group_check=True)

                # pipeline: prepare s+1 so PE has projection/PT work in flight
                # while DVE produces the next masks
                if s + 1 < NSC:
                    st = prep_sc(s + 1)

                # --- output: DVE evac; superchunks 1..30 pair up (two windows
                # per [125,512] tile, one DMA per pair) to halve DGE issues ---
                if s == 0:
                    wall_sb = spool.tile([125, 256], F32, name="wall_0",
                                         tag="wall0")
                    nc.vector.tensor_copy(wall_sb, wt)
                    nc.sync.dma_start(out=out[0:GP - B], in_=wall_sb[B:GP])
                elif s == NSC - 1:
                    wall_sb = spool.tile([125, 256], F32, name=f"wall_{s}",
                                         tag="wallz")
                    nc.vector.tensor_copy(wall_sb, wt)
                    nc.sync.dma_start(out=out[s * GP - B: s * GP - B + GP],
                                      in_=wall_sb)
                else:
                    half = (s - 1) % 2
                    if half == 0:
                        wpair["t"] = spool.tile([125, 512], F32,
                                                name=f"wallp_{s}", tag="wallp",
                                                bufs=2)
                    wp = wpair["t"]
                    nc.vector.tensor_copy(wp[:, half * 256:half * 256 + 256], wt)
                    if half == 1:
                        base = (s - 1) * GP - B
                        nc.sync.dma_start(
                            out=out[base: base + 2 * GP].rearrange(
                                "(a p) d -> p a d", a=2),
                            in_=wp)
                prev_mp, prev_v = mp_sb, v_sb

            # final output chunk 799 = intra tail of the last superchunk
            wtf = psW.tile([125, 256], F32, name="wt_final", tag="wt")
            mm(wtf, prev_mp[:, 250:375], prev_v, start=True, stop=True)
            wallf_sb = spool.tile([5, 256], F32, name="wallf", tag="wallf")
            nc.vector.tensor_copy(wallf_sb, wtf[0:5])
            nc.sync.dma_start(out=out[SEQ - B:SEQ], in_=wallf_sb)

    return nc


def _col_scales():
    j = np.arange(SEQ) // B          # global chunk index
    sq = (np.float64(g6) ** j).astype(np.float32)
    sk = (np.float64(g6) ** (-j)).astype(np.float32)
    return sq, sk


def prep_core_inputs(xq2d, xk2d, xv2d, wqkv):
    sq, sk = _col_scales()
    bf = ml_dtypes.bfloat16
    return {
        "xqT": np.ascontiguousarray((xq2d.T * sq[None, :]).astype(bf)),
        "xkT": np.ascontiguousarray((xk2d.T * sk[None, :]).astype(bf)),
        "xvT": np.ascontiguousarray(xv2d.T.astype(bf)),
        "wqkv": wqkv,
    }


def make_in_maps(inputs):
    """inputs: dict from setup_inputs (full batch). Returns per-core in_maps."""
    xq, xk, xv = inputs["xq"], inputs["xk"], inputs["xv"]
    wqkv = np.ascontiguousarray(np.concatenate(
        [np.asarray(inputs["Wq"], dtype=np.float32),
         np.asarray(inputs["Wk"], dtype=np.float32),
         np.asarray(inputs["Wv"], dtype=np.float32)],
        axis=1).astype(ml_dtypes.bfloat16))
    in_maps = []
    for b in range(8):
        in_maps.append(prep_core_inputs(
            np.asarray(xq[b], dtype=np.float32),
            np.asarray(xk[b], dtype=np.float32),
            np.asarray(xv[b], dtype=np.float32), wqkv))
    return in_maps


_NC_CACHE = {}


def _get_nc():
    if "nc" not in _NC_CACHE:
        from concourse import bacc
        nc = bacc.Bacc("TRN2", target_bir_lowering=False, debug=False)
        build_kernel(nc)
        nc.compile()
        _NC_CACHE["nc"] = nc
    return _NC_CACHE["nc"]


def run(inputs, trace=False, **kwargs):
    """Run on 8 NeuronCores; returns (output [8,4000,256], BassKernelResults)."""
    from concourse.bass_utils import run_bass_kernel_spmd

    nc = _get_nc()
    in_maps = make_in_maps(inputs)
    res = run_bass_kernel_spmd(nc, in_maps, core_ids=list(range(8)),
                               trace=trace, **kwargs)
    out = np.stack([r["out"] for r in res.results], axis=0)
    return out, res


def kernel(**inputs) -> np.ndarray:
    out, _ = run(inputs)
    return out


# revision 31
# speedup vs baseline: 1.1823x; 1.0272x over previous
"""Bass/Tile kernel for chunkwise retention (nn_ChunkwiseRetention).

Algorithm (per core = one batch element, seq 4000, B=5, 800 chunks):
superchunks of G=25 chunks (125 positions). The host pre-scales
xqT columns by g6^j and xkT by g6^-j (j = global chunk index), which
folds the entire cross-chunk decay into the projections: the cross
mask becomes 0/1, the carry is Q~ @ U with no rescale, and the state
update needs no scaling at all.

Per superchunk s: Q~^T,K~^T (dim-major, projected 4 superchunks at a
time at N=500) and K~,V (pos-major) projections; P~^T = K~ @ Q~^T;
masked matmuls accumulate cross + intra (+5-row shift via
free-dim-shifted stationary) + seam (previous superchunk's tail
stationary x previous V) + carry (Q~ @ U) into one PSUM window;
running state U in one PSUM bank (zero-matmul init, per-element
has_written accumulation).

All matmul operands are bf16 (host casts the scaled inputs): 1
cycle/row on the PE at any moving size (f32r needs moving>=256, f32 is
4 cycles/row), so the P~^T matmuls (N=125) run 4x faster than f32.
bf16 keeps f32's exponent range, which the g6^+-j scaling needs.
PSUM accumulation stays f32; the final output is written f32.

Engine split (GPSIMD cannot touch PSUM, so evacuations go DVE/ACT):
DVE P~^T evac + kv evac + wall copy; ACT qt/kt (single 2-segment ops)
+ ut; Pool (GPSIMD) the fused mask-multiply (SBUF-only). K~ pos-major
comes from PE transposes of K~^T into the kv bank (saves 256 PE
rows/superchunk vs projecting K twice). Output DMAs are paired (two
superchunks per DMA) to halve SP DGE issue pressure.

PSUM banks (8): qkt(shared) 2 + kv 2 + pt 1 + wt 2 + u 1.
"""
import numpy as np
import ml_dtypes

import concourse.bass as bass
import concourse.mybir as mybir
import concourse.tile as tile

GAMMA = 0.9865
B = 5
SEQ = 4000
FEAT = 256
DIM = 256
G = 25
GP = G * B            # 125
NSC = SEQ // GP       # 32
LG = 4                # superchunks per projection/load group
LGP = LG * GP         # 500
F32 = mybir.dt.float32
F32R = mybir.dt.float32r
BF16 = mybir.dt.bfloat16
g6 = float(np.float64(GAMMA) ** 6)
COPY = mybir.ActivationFunctionType.Copy

# const blob column layout (f32 masks for the DVE multiplies). wct and wit
# sit 130 columns apart so ONE fused DVE tensor_mul (2-segment AP, outer
# stride 130) produces both masked stationaries from a double-read of P~^T.
C_WCT = 0            # [0:125)   0/1 strict lower-block-triangular cross mask
C_WIT = 130          # [130:255) intra decay mask (rows 0:125)
C_END = 260          # padded so the 2-segment (stride 130) view fits


def make_const_blob():
    t = np.arange(GP) // B
    p = np.arange(GP) % B
    tb, ta = t[:, None], t[None, :]
    wct01 = (tb < ta).astype(np.float32)
    qb, pa = p[:, None], p[None, :]
    wit = np.where((tb == ta) & (pa >= qb),
                   np.float64(GAMMA) ** (qb - pa), 0.0).astype(np.float32)
    blob = np.zeros((128, C_END), np.float32)
    blob[0:GP, C_WCT:C_WCT + 125] = wct01
    blob[0:GP, C_WIT:C_WIT + 125] = wit
    return blob


def build_kernel(nc: bass.Bass):
    xqT = nc.dram_tensor("xqT", [FEAT, SEQ], BF16, kind="ExternalInput").ap()
    xkT = nc.dram_tensor("xkT", [FEAT, SEQ], BF16, kind="ExternalInput").ap()
    xvT = nc.dram_tensor("xvT", [FEAT, SEQ], BF16, kind="ExternalInput").ap()
    wqkv = nc.dram_tensor("wqkv", [FEAT, 3 * DIM], BF16, kind="ExternalInput").ap()
    out = nc.dram_tensor("out", [SEQ, DIM], F32, kind="ExternalOutput").ap()

    blob_np = make_const_blob()
    mm = nc.tensor.matmul

    with tile.TileContext(nc) as tc:
        with (
            tc.tile_pool(name="consts", bufs=1) as cpool,
            tc.tile_pool(name="xin", bufs=2) as xpool,
            tc.tile_pool(name="work", bufs=2) as spool,
            tc.tile_pool(name="psT", bufs=1, space="PSUM") as psT,
            tc.tile_pool(name="psP", bufs=2, space="PSUM") as psP,
            tc.tile_pool(name="psPT", bufs=1, space="PSUM") as psPT,
            tc.tile_pool(name="psW", bufs=2, space="PSUM") as psW,
            tc.tile_pool(name="psU", bufs=1, space="PSUM") as psU,
        ):
            # --- constants to SBUF. The weights DMA is split wk/wq/wv (ACT
            # DGE queue, K first) so the K projection — the first real PE
            # work — starts as early as possible; x/blob/identity go on SP
            # in first-use order so both queues fill in parallel. ---
            w_sb = cpool.tile([128, 2, 3 * DIM], BF16, name="w_sb")
            wr = wqkv.rearrange("(h p) d -> p h d", p=128)
            nc.scalar.dma_start(out=w_sb[:, :, 256:512], in_=wr[:, :, 256:512])
            nc.scalar.dma_start(out=w_sb[:, :, 0:256], in_=wr[:, :, 0:256])
            nc.scalar.dma_start(out=w_sb[:, :, 512:768], in_=wr[:, :, 512:768])
            blob_sb = cpool.tile([128, C_END], F32, name="blob_sb")
            nc.sync.dma_start(out=blob_sb,
                              in_=nc.inline_tensor(blob_np, "cblob").ap())
            idn_np = np.eye(128, dtype=np.float32).astype(ml_dtypes.bfloat16)
            idn_sb = cpool.tile([128, 128], BF16, name="idn_sb")
            nc.sync.dma_start(out=idn_sb,
                              in_=nc.inline_tensor(idn_np, "idn").ap())
            wv_sb = w_sb[:, :, 512:768]

            u_ps = psU.tile([128, 512], F32, name="u_state")

            # persistent combined mask stationaries (manual quad-buffer for
            # the 2-superchunk lookahead): cols 0:125 = mpc (cross, rewritten
            # fully each superchunk); cols 125:375 = mpi region (shift trick:
            # write 130:255, main read 125:250, tail read 250:375); zeros
            # memset once. One fused op writes both segments (stride 130).
            mp_bufs = []
            for i_ in range(4):
                mb_ = spool.tile([125, 375], BF16, name=f"mp_{i_}", tag=f"mp_{i_}",
                                 bufs=1)
                nc.vector.memset(mb_[:, 125:130], 0.0)
                nc.vector.memset(mb_[:, 255:375], 0.0)
                mp_bufs.append(mb_)

            prev_mp = prev_v = None
            xg = {}
            qkt_sb = {}
            wpair = {}

            def load_x(gidx):
                gsl = slice(gidx * LGP, (gidx + 1) * LGP)
                xq_g = xpool.tile([128, 2, LGP], BF16, name=f"xq_{gidx}", tag="xq")
                xk_g = xpool.tile([128, 2, LGP], BF16, name=f"xk_{gidx}", tag="xk")
                xv_g = xpool.tile([128, 2, LGP], BF16, name=f"xv_{gidx}", tag="xv")
                # group 0 spreads across both DGE queues to cut startup latency
                eng_q = nc.scalar if gidx == 0 else nc.sync
                nc.sync.dma_start(out=xq_g, in_=xqT[:, gsl].rearrange("(h p) a -> p h a", p=128))
                eng_q.dma_start(out=xk_g, in_=xkT[:, gsl].rearrange("(h p) a -> p h a", p=128))
                nc.sync.dma_start(out=xv_g, in_=xvT[:, gsl].rearrange("(h p) a -> p h a", p=128))
                xg[gidx] = (xq_g, xk_g, xv_g)

            def project_group(gidx):
                """Q~^T / K~^T projections for the group (N=500), via a shared
                psum tag: d-lo cols 0:500 (bank 0), d-hi cols 512:1012
                (bank 1), each bank one closed accumulation group. K first so
                its evac (needed by the boundary transpose + P~^T) overlaps
                the Q projection."""
                xq_g, xk_g, xv_g = xg[gidx]
                qt_sb = spool.tile([128, 1000], BF16, name=f"qt_{gidx}", tag="qt")
                kt_sb = spool.tile([128, 1000], BF16, name=f"kt_{gidx}", tag="kt")
                qkt_k = psT.tile([128, 1024], F32, name=f"qkt_k_{gidx}", tag="qkt")
                for off, dlo in ((0, 256), (512, 384)):
                    for h in (0, 1):
                        mm(qkt_k[:, off:off + LGP], w_sb[:, h, dlo:dlo + 128],
                           xk_g[:, h, :], start=(h == 0), stop=(h == 1))
                nc.scalar.activation(
                    kt_sb.rearrange("p (a b) -> p a b", b=500),
                    qkt_k.rearrange("p (a b) -> p a b", b=512)[:, :, 0:500], COPY)
                qkt_q = psT.tile([128, 1024], F32, name=f"qkt_q_{gidx}", tag="qkt")
                for off, dlo in ((0, 0), (512, 128)):
                    for h in (0, 1):
                        mm(qkt_q[:, off:off + LGP], w_sb[:, h, dlo:dlo + 128],
                           xq_g[:, h, :], start=(h == 0), stop=(h == 1))
                nc.scalar.activation(
                    qt_sb.rearrange("p (a b) -> p a b", b=500),
                    qkt_q.rearrange("p (a b) -> p a b", b=512)[:, :, 0:500], COPY)
                qkt_sb[gidx] = (qt_sb, kt_sb)

            def prep_sc(s):
                """K~/V + P~^T + masked stationaries for superchunk s.
                Group g+1's x-load fires at ls==1 and its projections at
                ls==2, two superchunks before the boundary, so the boundary
                never waits on the qt/kt evacuations."""
                gidx, ls = divmod(s, LG)
                if ls == 1 and gidx + 1 < NSC // LG:
                    load_x(gidx + 1)
                if ls == 2 and gidx + 1 < NSC // LG:
                    project_group(gidx + 1)
                    xg.pop(gidx - 1, None)
                    qkt_sb.pop(gidx - 1, None)
                xq_g, xk_g, xv_g = xg[gidx]
                qt_sb, kt_sb = qkt_sb[gidx]
                lsl = slice(ls * GP, (ls + 1) * GP)
                xv_s = xv_g[:, :, lsl]
                qlo = qt_sb[:, ls * GP:(ls + 1) * GP]
                qhi = qt_sb[:, 500 + ls * GP:500 + (ls + 1) * GP]
                klo = kt_sb[:, ls * GP:(ls + 1) * GP]
                khi = kt_sb[:, 500 + ls * GP:500 + (ls + 1) * GP]

                # K~ pos-major via PE transpose of K~^T (bf16, written into
                # the f32 kv bank's first 512B via bitcast views); V pos-major
                # projected from x (f32 cols 256:512)
                kv = psP.tile([125, 512], F32, name=f"kv_{s}", tag="kv")
                nc.tensor.transpose(kv[:, 0:64].bitcast(BF16), klo, idn_sb)
                nc.tensor.transpose(kv[:, 64:128].bitcast(BF16), khi, idn_sb)
                for h in (0, 1):
                    mm(kv[:, 256:512], xv_s[:, h, :], wv_sb[:, h, :],
                       start=(h == 0), stop=(h == 1))
                kv_sb = spool.tile([125, 512], BF16, name=f"kv_sb_{s}", tag="kvsb", bufs=4)
                nc.vector.tensor_copy(kv_sb[:, 0:256], kv[:, 0:128].bitcast(BF16))
                nc.vector.tensor_copy(kv_sb[:, 256:512], kv[:, 256:512])
                k_sb = kv_sb[:, 0:256]
                v_sb = kv_sb[:, 256:512]

                # P~^T = K~ @ Q~^T (bf16: 1 cycle/row at any N)
                pt_ps = psPT.tile([125, 125], F32, name=f"pt_{s}", tag="pt")
                mm(pt_ps, klo, qlo, start=True, stop=False)
                mm(pt_ps, khi, qhi, start=False, stop=True)
                pt_sb = spool.tile([125, 125], F32, name=f"pt_sb_{s}", tag="ptsb",
                                   bufs=3)
                nc.vector.tensor_copy(pt_sb, pt_ps)

                # one fused GPSIMD op: {mpc (cols 0:125), shifted mpi
                # (130:255)} = P~^T (read twice, stride-0) * {wct, wit}
                mp_sb = mp_bufs[s % 4]
                mp_out = mp_sb[:, 0:260].rearrange(
                    "p (a b) -> p a b", b=130)[:, :, 0:125]
                masks_in = blob_sb[0:GP, C_WCT:C_WCT + 260].rearrange(
                    "p (a b) -> p a b", b=130)[:, :, 0:125]
                pt_in = pt_sb.unsqueeze(1).broadcast_to((GP, 2, 125))
                nc.gpsimd.tensor_mul(mp_out, pt_in, masks_in)
                return dict(k_sb=k_sb, v_sb=v_sb, mp_sb=mp_sb, qlo=qlo, qhi=qhi)

            load_x(0)
            project_group(0)
            sts = {0: prep_sc(0), 1: prep_sc(1)}
            for s in range(NSC):
                st = sts.pop(s)
                k_sb, v_sb = st["k_sb"], st["v_sb"]
                mp_sb = st["mp_sb"]
                qlo, qhi = st["qlo"], st["qhi"]

                # --- window accumulation (one closed group). The seam
                # (intra tail of chunk s*G-1) is added directly from the
                # previous superchunk's tail stationary and V: rows 5:125 of
                # that matmul multiply zero columns and accumulate zeros. ---
                wt = psW.tile([125, 256], F32, name=f"wt_{s}", tag="wt")
                mm(wt, mp_sb[:, 0:125], v_sb, start=True, stop=False)
                mm(wt, mp_sb[:, 125:250], v_sb, start=False, stop=(s == 0))
                if s > 0:
                    ut_sb = spool.tile([128, 512], BF16, name=f"ut_{s}", tag="ut")
                    nc.scalar.activation(ut_sb, u_ps, COPY)
                    mm(wt, prev_mp[:, 250:375], prev_v, start=False, stop=False)
                    mm(wt, qlo, ut_sb[:, 0:256], start=False, stop=False)
                    mm(wt, qhi, ut_sb[:, 256:512], start=False, stop=True)

                # --- state update (s=0 opens the accumulation bank) ---
                mm(u_ps[:, 0:256], k_sb[:, 0:128], v_sb,
                   start=(s == 0), stop=True, skip_group_check=True)
                mm(u_ps[:, 256:512], k_sb[:, 128:256], v_sb,
                   start=(s == 0), stop=True, skip_group_check=True)

                # pipeline: prepare s+2 (two superchunks of lookahead) so the
                # P~^T -> evac -> GPSIMD mask chain and the kv/qt/kt
                # evacuations are never on the PE's critical path
                if s + 2 < NSC:
                    sts[s + 2] = prep_sc(s + 2)

                # --- output: DVE evac; superchunks 1..30 pair up (two windows
                # per [125,512] tile, one DMA per pair) to halve DGE issues ---
                if s == 0:
                    wall_sb = spool.tile([125, 256], F32, name="wall_0",
                                         tag="wall0")
                    nc.vector.tensor_copy(wall_sb, wt)
                    nc.sync.dma_start(out=out[0:GP - B], in_=wall_sb[B:GP])
                elif s == NSC - 1:
                    wall_sb = spool.tile([125, 256], F32, name=f"wall_{s}",
                                         tag="wallz")
                    nc.vector.tensor_copy(wall_sb, wt)
                    nc.scalar.dma_start(out=out[s * GP - B: s * GP - B + GP],
                                        in_=wall_sb)
                else:
                    half = (s - 1) % 2
                    if half == 0:
                        wpair["t"] = spool.tile([125, 512], F32,
                                                name=f"wallp_{s}", tag="wallp",
                                                bufs=2)
                    wp = wpair["t"]
                    nc.vector.tensor_copy(wp[:, half * 256:half * 256 + 256], wt)
                    if half == 1:
                        base = (s - 1) * GP - B
                        nc.sync.dma_start(
                            out=out[base: base + 2 * GP].rearrange(
                                "(a p) d -> p a d", a=2),
                            in_=wp)
                prev_mp, prev_v = mp_sb, v_sb

            # final output chunk 799 = intra tail of the last superchunk
            wtf = psW.tile([125, 256], F32, name="wt_final", tag="wt")
            mm(wtf, prev_mp[:, 250:375], prev_v, start=True, stop=True)
            wallf_sb = spool.tile([5, 256], F32, name="wallf", tag="wallf")
            nc.vector.tensor_copy(wallf_sb, wtf[0:5])
            nc.sync.dma_start(out=out[SEQ - B:SEQ], in_=wallf_sb)

    return nc


def _col_scales():
    j = np.arange(SEQ) // B          # global chunk index
    sq = (np.float64(g6) ** j).astype(np.float32)
    sk = (np.float64(g6) ** (-j)).astype(np.float32)
    return sq, sk


def prep_core_inputs(xq2d, xk2d, xv2d, wqkv):
    sq, sk = _col_scales()
    bf = ml_dtypes.bfloat16
    return {
        "xqT": np.ascontiguousarray((xq2d.T * sq[None, :]).astype(bf)),
        "xkT": np.ascontiguousarray((xk2d.T * sk[None, :]).astype(bf)),
        "xvT": np.ascontiguousarray(xv2d.T.astype(bf)),
        "wqkv": wqkv,
    }


def make_in_maps(inputs):
    """inputs: dict from setup_inputs (full batch). Returns per-core in_maps."""
    xq, xk, xv = inputs["xq"], inputs["xk"], inputs["xv"]
    wqkv = np.ascontiguousarray(np.concatenate(
        [np.asarray(inputs["Wq"], dtype=np.float32),
         np.asarray(inputs["Wk"], dtype=np.float32),
         np.asarray(inputs["Wv"], dtype=np.float32)],
        axis=1).astype(ml_dtypes.bfloat16))
    in_maps = []
    for b in range(8):
        in_maps.append(prep_core_inputs(
            np.asarray(xq[b], dtype=np.float32),
            np.asarray(xk[b], dtype=np.float32),
            np.asarray(xv[b], dtype=np.float32), wqkv))
    return in_maps


_NC_CACHE = {}


def _get_nc():
    if "nc" not in _NC_CACHE:
        from concourse import bacc
        nc = bacc.Bacc("TRN2", target_bir_lowering=False, debug=False)
        build_kernel(nc)
        nc.compile()
        _NC_CACHE["nc"] = nc
    return _NC_CACHE["nc"]


def run(inputs, trace=False, **kwargs):
    """Run on 8 NeuronCores; returns (output [8,4000,256], BassKernelResults)."""
    from concourse.bass_utils import run_bass_kernel_spmd

    nc = _get_nc()
    in_maps = make_in_maps(inputs)
    res = run_bass_kernel_spmd(nc, in_maps, core_ids=list(range(8)),
                               trace=trace, **kwargs)
    out = np.stack([r["out"] for r in res.results], axis=0)
    return out, res


def kernel(**inputs) -> np.ndarray:
    out, _ = run(inputs)
    return out


# revision 34
# speedup vs baseline: 1.1958x; 1.0114x over previous
"""Bass/Tile kernel for chunkwise retention (nn_ChunkwiseRetention).

Algorithm (per core = one batch element, seq 4000, B=5, 800 chunks):
superchunks of G=25 chunks (125 positions). The host pre-scales
xqT columns by g6^j and xkT by g6^-j (j = global chunk index), which
folds the entire cross-chunk decay into the projections: the cross
mask becomes 0/1, the carry is Q~ @ U with no rescale, and the state
update needs no scaling at all.

Per superchunk s: Q~^T,K~^T (dim-major, projected 4 superchunks at a
time at N=500) and K~,V (pos-major) projections; P~^T = K~ @ Q~^T;
masked matmuls accumulate cross + intra (+5-row shift via
free-dim-shifted stationary) + seam (previous superchunk's tail
stationary x previous V) + carry (Q~ @ U) into one PSUM window;
running state U in one PSUM bank (zero-matmul init, per-element
has_written accumulation).

All matmul operands are bf16 (host casts the scaled inputs): 1
cycle/row on the PE at any moving size (f32r needs moving>=256, f32 is
4 cycles/row), so the P~^T matmuls (N=125) run 4x faster than f32.
bf16 keeps f32's exponent range, which the g6^+-j scaling needs.
PSUM accumulation stays f32; the final output is written f32.

Engine split (GPSIMD cannot touch PSUM, so evacuations go DVE/ACT):
DVE P~^T evac + kv evac + wall copy; ACT qt/kt (single 2-segment ops)
+ ut; Pool (GPSIMD) the fused mask-multiply (SBUF-only). K~ pos-major
comes from PE transposes of K~^T into the kv bank (saves 256 PE
rows/superchunk vs projecting K twice). Output DMAs are paired (two
superchunks per DMA) to halve SP DGE issue pressure.

PSUM banks (8): qkt(shared) 2 + kv 2 + pt 1 + wt 2 + u 1.
"""
import numpy as np
import ml_dtypes

import concourse.bass as bass
import concourse.mybir as mybir
import concourse.tile as tile

GAMMA = 0.9865
B = 5
SEQ = 4000
FEAT = 256
DIM = 256
G = 25
GP = G * B            # 125
NSC = SEQ // GP       # 32
LG = 4                # superchunks per projection/load group
LGP = LG * GP         # 500
F32 = mybir.dt.float32
F32R = mybir.dt.float32r
BF16 = mybir.dt.bfloat16
g6 = float(np.float64(GAMMA) ** 6)
COPY = mybir.ActivationFunctionType.Copy

# const blob column layout (f32 masks for the DVE multiplies). wct and wit
# sit 130 columns apart so ONE fused DVE tensor_mul (2-segment AP, outer
# stride 130) produces both masked stationaries from a double-read of P~^T.
C_WCT = 0            # [0:125)   0/1 strict lower-block-triangular cross mask
C_WIT = 130          # [130:255) intra decay mask (rows 0:125)
C_END = 260          # padded so the 2-segment (stride 130) view fits


def make_const_blob():
    t = np.arange(GP) // B
    p = np.arange(GP) % B
    tb, ta = t[:, None], t[None, :]
    wct01 = (tb < ta).astype(np.float32)
    qb, pa = p[:, None], p[None, :]
    wit = np.where((tb == ta) & (pa >= qb),
                   np.float64(GAMMA) ** (qb - pa), 0.0).astype(np.float32)
    blob = np.zeros((128, C_END), np.float32)
    blob[0:GP, C_WCT:C_WCT + 125] = wct01
    blob[0:GP, C_WIT:C_WIT + 125] = wit
    return blob


def build_kernel(nc: bass.Bass):
    xqT = nc.dram_tensor("xqT", [FEAT, SEQ], BF16, kind="ExternalInput").ap()
    xkT = nc.dram_tensor("xkT", [FEAT, SEQ], BF16, kind="ExternalInput").ap()
    xvT = nc.dram_tensor("xvT", [FEAT, SEQ], BF16, kind="ExternalInput").ap()
    wqkv = nc.dram_tensor("wqkv", [FEAT, 3 * DIM], BF16, kind="ExternalInput").ap()
    out = nc.dram_tensor("out", [SEQ, DIM], F32, kind="ExternalOutput").ap()

    blob_np = make_const_blob()
    mm = nc.tensor.matmul

    with tile.TileContext(nc) as tc:
        with (
            tc.tile_pool(name="consts", bufs=1) as cpool,
            tc.tile_pool(name="xin", bufs=2) as xpool,
            tc.tile_pool(name="work", bufs=2) as spool,
            tc.tile_pool(name="psT", bufs=1, space="PSUM") as psT,
            tc.tile_pool(name="psP", bufs=2, space="PSUM") as psP,
            tc.tile_pool(name="psPT", bufs=1, space="PSUM") as psPT,
            tc.tile_pool(name="psW", bufs=2, space="PSUM") as psW,
            tc.tile_pool(name="psU", bufs=1, space="PSUM") as psU,
        ):
            # --- constants to SBUF. The weights DMA is split wk/wq/wv (ACT
            # DGE queue, K first) so the K projection — the first real PE
            # work — starts as early as possible; x/blob/identity go on SP
            # in first-use order so both queues fill in parallel. ---
            w_sb = cpool.tile([128, 2, 3 * DIM], BF16, name="w_sb")
            wr = wqkv.rearrange("(h p) d -> p h d", p=128)
            nc.scalar.dma_start(out=w_sb[:, :, 256:512], in_=wr[:, :, 256:512])
            nc.scalar.dma_start(out=w_sb[:, :, 0:256], in_=wr[:, :, 0:256])
            nc.scalar.dma_start(out=w_sb[:, :, 512:768], in_=wr[:, :, 512:768])
            blob_sb = cpool.tile([128, C_END], F32, name="blob_sb")
            nc.sync.dma_start(out=blob_sb,
                              in_=nc.inline_tensor(blob_np, "cblob").ap())
            idn_np = np.zeros((128, 640), np.float32)
            idn_np[:, 0:128] = np.eye(128, dtype=np.float32)
            idn_np = idn_np.astype(ml_dtypes.bfloat16)
            idn_full = cpool.tile([128, 640], BF16, name="idn_sb")
            nc.sync.dma_start(out=idn_full,
                              in_=nc.inline_tensor(idn_np, "idn").ap())
            idn_sb = idn_full[:, 0:128]
            wv_sb = w_sb[:, :, 512:768]

            u_ps = psU.tile([128, 512], F32, name="u_state")
            # zero-matmul initializes the U bank's data + has_written bits so
            # the per-superchunk state matmuls can all accumulate
            nc.tensor.matmul(u_ps, idn_full[0:1, 128:256],
                             idn_full[0:1, 128:640],
                             start=True, stop=True, skip_group_check=True)

            # persistent combined mask stationaries (manual quad-buffer for
            # the 2-superchunk lookahead): cols 0:125 = mpc (cross, rewritten
            # fully each superchunk); cols 125:375 = mpi region (shift trick:
            # write 130:255, main read 125:250, tail read 250:375); zeros
            # memset once. One fused op writes both segments (stride 130).
            mp_bufs = []
            for i_ in range(4):
                mb_ = spool.tile([125, 375], BF16, name=f"mp_{i_}", tag=f"mp_{i_}",
                                 bufs=1)
                nc.vector.memset(mb_[:, 125:130], 0.0)
                nc.vector.memset(mb_[:, 255:375], 0.0)
                mp_bufs.append(mb_)

            prev_mp = prev_v = None
            xg = {}
            qkt_sb = {}
            wpair = {}

            def load_x(gidx):
                gsl = slice(gidx * LGP, (gidx + 1) * LGP)
                xq_g = xpool.tile([128, 2, LGP], BF16, name=f"xq_{gidx}", tag="xq")
                xk_g = xpool.tile([128, 2, LGP], BF16, name=f"xk_{gidx}", tag="xk")
                xv_g = xpool.tile([128, 2, LGP], BF16, name=f"xv_{gidx}", tag="xv")
                # group 0 spreads across both DGE queues to cut startup latency
                eng_q = nc.scalar if gidx == 0 else nc.sync
                nc.sync.dma_start(out=xq_g, in_=xqT[:, gsl].rearrange("(h p) a -> p h a", p=128))
                eng_q.dma_start(out=xk_g, in_=xkT[:, gsl].rearrange("(h p) a -> p h a", p=128))
                nc.sync.dma_start(out=xv_g, in_=xvT[:, gsl].rearrange("(h p) a -> p h a", p=128))
                xg[gidx] = (xq_g, xk_g, xv_g)

            def project_group(gidx):
                """Q~^T / K~^T projections for the group (N=500), via a shared
                psum tag: d-lo cols 0:500 (bank 0), d-hi cols 512:1012
                (bank 1), each bank one closed accumulation group. K first so
                its evac (needed by the boundary transpose + P~^T) overlaps
                the Q projection."""
                xq_g, xk_g, xv_g = xg[gidx]
                qt_sb = spool.tile([128, 1000], BF16, name=f"qt_{gidx}", tag="qt")
                kt_sb = spool.tile([128, 1000], BF16, name=f"kt_{gidx}", tag="kt")
                qkt_k = psT.tile([128, 1024], F32, name=f"qkt_k_{gidx}", tag="qkt")
                for off, dlo in ((0, 256), (512, 384)):
                    for h in (0, 1):
                        mm(qkt_k[:, off:off + LGP], w_sb[:, h, dlo:dlo + 128],
                           xk_g[:, h, :], start=(h == 0), stop=(h == 1))
                nc.scalar.activation(
                    kt_sb.rearrange("p (a b) -> p a b", b=500),
                    qkt_k.rearrange("p (a b) -> p a b", b=512)[:, :, 0:500], COPY)
                qkt_q = psT.tile([128, 1024], F32, name=f"qkt_q_{gidx}", tag="qkt")
                for off, dlo in ((0, 0), (512, 128)):
                    for h in (0, 1):
                        mm(qkt_q[:, off:off + LGP], w_sb[:, h, dlo:dlo + 128],
                           xq_g[:, h, :], start=(h == 0), stop=(h == 1))
                nc.scalar.activation(
                    qt_sb.rearrange("p (a b) -> p a b", b=500),
                    qkt_q.rearrange("p (a b) -> p a b", b=512)[:, :, 0:500], COPY)
                qkt_sb[gidx] = (qt_sb, kt_sb)

            def prep_sc(s):
                """K~/V + P~^T + masked stationaries for superchunk s.
                Group g+1's x-load fires at ls==1 and its projections at
                ls==2, two superchunks before the boundary, so the boundary
                never waits on the qt/kt evacuations."""
                gidx, ls = divmod(s, LG)
                if ls == 1 and gidx + 1 < NSC // LG:
                    load_x(gidx + 1)
                if ls == 2 and gidx + 1 < NSC // LG:
                    project_group(gidx + 1)
                    xg.pop(gidx - 1, None)
                    qkt_sb.pop(gidx - 1, None)
                xq_g, xk_g, xv_g = xg[gidx]
                qt_sb, kt_sb = qkt_sb[gidx]
                lsl = slice(ls * GP, (ls + 1) * GP)
                xv_s = xv_g[:, :, lsl]
                qlo = qt_sb[:, ls * GP:(ls + 1) * GP]
                qhi = qt_sb[:, 500 + ls * GP:500 + (ls + 1) * GP]
                klo = kt_sb[:, ls * GP:(ls + 1) * GP]
                khi = kt_sb[:, 500 + ls * GP:500 + (ls + 1) * GP]

                # K~ pos-major via PE transpose of K~^T (bf16, written into
                # the f32 kv bank's first 512B via bitcast views); V pos-major
                # projected from x (f32 cols 256:512)
                kv = psP.tile([125, 512], F32, name=f"kv_{s}", tag="kv")
                nc.tensor.transpose(kv[:, 0:64].bitcast(BF16), klo, idn_sb)
                nc.tensor.transpose(kv[:, 64:128].bitcast(BF16), khi, idn_sb)
                for h in (0, 1):
                    mm(kv[:, 256:512], xv_s[:, h, :], wv_sb[:, h, :],
                       start=(h == 0), stop=(h == 1))
                kv_sb = spool.tile([125, 512], BF16, name=f"kv_sb_{s}", tag="kvsb", bufs=4)
                nc.vector.tensor_copy(kv_sb[:, 0:256], kv[:, 0:128].bitcast(BF16))
                nc.vector.tensor_copy(kv_sb[:, 256:512], kv[:, 256:512])
                k_sb = kv_sb[:, 0:256]
                v_sb = kv_sb[:, 256:512]

                # P~^T = K~ @ Q~^T (bf16: 1 cycle/row at any N)
                pt_ps = psPT.tile([125, 125], F32, name=f"pt_{s}", tag="pt")
                mm(pt_ps, klo, qlo, start=True, stop=False)
                mm(pt_ps, khi, qhi, start=False, stop=True)
                pt_sb = spool.tile([125, 125], F32, name=f"pt_sb_{s}", tag="ptsb",
                                   bufs=3)
                nc.vector.tensor_copy(pt_sb, pt_ps)

                # one fused GPSIMD op: {mpc (cols 0:125), shifted mpi
                # (130:255)} = P~^T (read twice, stride-0) * {wct, wit}
                mp_sb = mp_bufs[s % 4]
                mp_out = mp_sb[:, 0:260].rearrange(
                    "p (a b) -> p a b", b=130)[:, :, 0:125]
                masks_in = blob_sb[0:GP, C_WCT:C_WCT + 260].rearrange(
                    "p (a b) -> p a b", b=130)[:, :, 0:125]
                pt_in = pt_sb.unsqueeze(1).broadcast_to((GP, 2, 125))
                nc.gpsimd.tensor_mul(mp_out, pt_in, masks_in)
                return dict(k_sb=k_sb, v_sb=v_sb, mp_sb=mp_sb, qlo=qlo, qhi=qhi)

            load_x(0)
            project_group(0)
            sts = {0: prep_sc(0), 1: prep_sc(1)}
            for s in range(NSC):
                st = sts.pop(s)
                k_sb, v_sb = st["k_sb"], st["v_sb"]
                mp_sb = st["mp_sb"]
                qlo, qhi = st["qlo"], st["qhi"]

                # --- window accumulation (one closed group). The seam
                # (intra tail of chunk s*G-1) is added directly from the
                # previous superchunk's tail stationary and V: rows 5:125 of
                # that matmul multiply zero columns and accumulate zeros. ---
                wt = psW.tile([125, 256], F32, name=f"wt_{s}", tag="wt")
                mm(wt, mp_sb[:, 0:125], v_sb, start=True, stop=False)
                mm(wt, mp_sb[:, 125:250], v_sb, start=False, stop=(s == 0))
                if s > 0:
                    ut_sb = spool.tile([128, 512], BF16, name=f"ut_{s}", tag="ut")
                    nc.scalar.activation(ut_sb, u_ps, COPY)
                    mm(wt, prev_mp[:, 250:375], prev_v, start=False, stop=False)
                    mm(wt, qlo, ut_sb[:, 0:256], start=False, stop=False)
                    mm(wt, qhi, ut_sb[:, 256:512], start=False, stop=True)

                # --- state update (accumulates; U bank bits set by zero-mm) ---
                mm(u_ps[:, 0:256], k_sb[:, 0:128], v_sb,
                   start=False, stop=True, skip_group_check=True)
                mm(u_ps[:, 256:512], k_sb[:, 128:256], v_sb,
                   start=False, stop=True, skip_group_check=True)

                # pipeline: prepare s+2 (two superchunks of lookahead) so the
                # P~^T -> evac -> GPSIMD mask chain and the kv/qt/kt
                # evacuations are never on the PE's critical path
                if s + 2 < NSC:
                    sts[s + 2] = prep_sc(s + 2)

                # --- output: DVE evac; superchunks 1..30 pair up (two windows
                # per [125,512] tile, one DMA per pair) to halve DGE issues ---
                if s == 0:
                    wall_sb = spool.tile([125, 256], F32, name="wall_0",
                                         tag="wall0")
                    nc.vector.tensor_copy(wall_sb, wt)
                    nc.sync.dma_start(out=out[0:GP - B], in_=wall_sb[B:GP])
                elif s == NSC - 1:
                    wall_sb = spool.tile([125, 256], F32, name=f"wall_{s}",
                                         tag="wallz")
                    nc.vector.tensor_copy(wall_sb, wt)
                    nc.scalar.dma_start(out=out[s * GP - B: s * GP - B + GP],
                                        in_=wall_sb)
                else:
                    half = (s - 1) % 2
                    if half == 0:
                        wpair["t"] = spool.tile([125, 512], F32,
                                                name=f"wallp_{s}", tag="wallp",
                                                bufs=2)
                    wp = wpair["t"]
                    nc.vector.tensor_copy(wp[:, half * 256:half * 256 + 256], wt)
                    if half == 1:
                        base = (s - 1) * GP - B
                        nc.sync.dma_start(
                            out=out[base: base + 2 * GP].rearrange(
                                "(a p) d -> p a d", a=2),
                            in_=wp)
                prev_mp, prev_v = mp_sb, v_sb

            # final output chunk 799 = intra tail of the last superchunk
            wtf = psW.tile([125, 256], F32, name="wt_final", tag="wt")
            mm(wtf, prev_mp[:, 250:375], prev_v, start=True, stop=True)
            wallf_sb = spool.tile([5, 256], F32, name="wallf", tag="wallf")
            nc.vector.tensor_copy(wallf_sb, wtf[0:5])
            nc.sync.dma_start(out=out[SEQ - B:SEQ], in_=wallf_sb)

    return nc


def _col_scales():
    j = np.arange(SEQ) // B          # global chunk index
    sq = (np.float64(g6) ** j).astype(np.float32)
    sk = (np.float64(g6) ** (-j)).astype(np.float32)
    return sq, sk


def prep_core_inputs(xq2d, xk2d, xv2d, wqkv):
    sq, sk = _col_scales()
    bf = ml_dtypes.bfloat16
    return {
        "xqT": np.ascontiguousarray((xq2d.T * sq[None, :]).astype(bf)),
        "xkT": np.ascontiguousarray((xk2d.T * sk[None, :]).astype(bf)),
        "xvT": np.ascontiguousarray(xv2d.T.astype(bf)),
        "wqkv": wqkv,
    }


def make_in_maps(inputs):
    """inputs: dict from setup_inputs (full batch). Returns per-core in_maps."""
    xq, xk, xv = inputs["xq"], inputs["xk"], inputs["xv"]
    wqkv = np.ascontiguousarray(np.concatenate(
        [np.asarray(inputs["Wq"], dtype=np.float32),
         np.asarray(inputs["Wk"], dtype=np.float32),
         np.asarray(inputs["Wv"], dtype=np.float32)],
        axis=1).astype(ml_dtypes.bfloat16))
    in_maps = []
    for b in range(8):
        in_maps.append(prep_core_inputs(
            np.asarray(xq[b], dtype=np.float32),
            np.asarray(xk[b], dtype=np.float32),
            np.asarray(xv[b], dtype=np.float32), wqkv))
    return in_maps


_NC_CACHE = {}


def _get_nc():
    if "nc" not in _NC_CACHE:
        from concourse import bacc
        nc = bacc.Bacc("TRN2", target_bir_lowering=False, debug=False)
        build_kernel(nc)
        nc.compile()
        _NC_CACHE["nc"] = nc
    return _NC_CACHE["nc"]


def run(inputs, trace=False, **kwargs):
    """Run on 8 NeuronCores; returns (output [8,4000,256], BassKernelResults)."""
    from concourse.bass_utils import run_bass_kernel_spmd

    nc = _get_nc()
    in_maps = make_in_maps(inputs)
    res = run_bass_kernel_spmd(nc, in_maps, core_ids=list(range(8)),
                               trace=trace, **kwargs)
    out = np.stack([r["out"] for r in res.results], axis=0)
    return out, res


def kernel(**inputs) -> np.ndarray:
    out, _ = run(inputs)
    return out


# revision 39
# speedup vs baseline: 1.2399x; 1.0369x over previous
"""Bass/Tile kernel for chunkwise retention (nn_ChunkwiseRetention).

Algorithm (per core = one batch element, seq 4000, B=5, 800 chunks):
superchunks of G=25 chunks (125 positions). The host pre-scales
xqT columns by g6^j and xkT by g6^-j (j = global chunk index), which
folds the entire cross-chunk decay into the projections: the cross
mask becomes 0/1, the carry is Q~ @ U with no rescale, and the state
update needs no scaling at all.

Per superchunk s: Q~^T,K~^T (dim-major, projected 4 superchunks at a
time at N=500) and K~,V (pos-major) projections; P~^T = K~ @ Q~^T;
masked matmuls accumulate cross + intra (+5-row shift via
free-dim-shifted stationary) + seam (previous superchunk's tail
stationary x previous V) + carry (Q~ @ U) into one PSUM window;
running state U in one PSUM bank (zero-matmul init, per-element
has_written accumulation).

All matmul operands are bf16 (host casts the scaled inputs): 1
cycle/row on the PE at any moving size (f32r needs moving>=256, f32 is
4 cycles/row), so the P~^T matmuls (N=125) run 4x faster than f32.
bf16 keeps f32's exponent range, which the g6^+-j scaling needs.
PSUM accumulation stays f32; the final output is written f32.

Engine split (GPSIMD cannot touch PSUM, so evacuations go DVE/ACT):
DVE P~^T evac + kv evac + wall copy; ACT qt/kt (single 2-segment ops)
+ ut; Pool (GPSIMD) the fused mask-multiply (SBUF-only). K~ pos-major
comes from PE transposes of K~^T into the kv bank (saves 256 PE
rows/superchunk vs projecting K twice). Output DMAs are paired (two
superchunks per DMA) to halve SP DGE issue pressure.

PSUM banks (8): qkt(shared) 2 + kv 2 + pt 1 + wt 2 + u 1.
"""
import numpy as np
import ml_dtypes

import concourse.bass as bass
import concourse.mybir as mybir
import concourse.tile as tile

GAMMA = 0.9865
B = 5
SEQ = 4000
FEAT = 256
DIM = 256
G = 25
GP = G * B            # 125
NSC = SEQ // GP       # 32
LG = 4                # superchunks per projection/load group
LGP = LG * GP         # 500
F32 = mybir.dt.float32
F32R = mybir.dt.float32r
BF16 = mybir.dt.bfloat16
g6 = float(np.float64(GAMMA) ** 6)
COPY = mybir.ActivationFunctionType.Copy

# const blob column layout (f32 masks for the DVE multiplies). wct and wit
# sit 130 columns apart so ONE fused DVE tensor_mul (2-segment AP, outer
# stride 130) produces both masked stationaries from a double-read of P~^T.
C_WCT = 0            # [0:125)   0/1 strict lower-block-triangular cross mask
C_WIT = 130          # [130:255) intra decay mask (rows 0:125)
C_END = 260          # padded so the 2-segment (stride 130) view fits


def make_const_blob():
    """One bf16 const: masks [0:260), 128x128 identity [260:388),
    zeros [388:900) (zero-matmul operands for the U-bank init)."""
    t = np.arange(GP) // B
    p = np.arange(GP) % B
    tb, ta = t[:, None], t[None, :]
    wct01 = (tb < ta).astype(np.float32)
    qb, pa = p[:, None], p[None, :]
    wit = np.where((tb == ta) & (pa >= qb),
                   np.float64(GAMMA) ** (qb - pa), 0.0).astype(np.float32)
    blob = np.zeros((128, 900), np.float32)
    blob[0:GP, C_WCT:C_WCT + 125] = wct01
    blob[0:GP, C_WIT:C_WIT + 125] = wit
    blob[:, 260:388] = np.eye(128, dtype=np.float32)
    return blob.astype(ml_dtypes.bfloat16)


def build_kernel(nc: bass.Bass):
    xqT = nc.dram_tensor("xqT", [FEAT, SEQ], BF16, kind="ExternalInput").ap()
    xkT = nc.dram_tensor("xkT", [FEAT, SEQ], BF16, kind="ExternalInput").ap()
    xvT = nc.dram_tensor("xvT", [FEAT, SEQ], BF16, kind="ExternalInput").ap()
    wqkv = nc.dram_tensor("wqkv", [FEAT, 3 * DIM], BF16, kind="ExternalInput").ap()
    out = nc.dram_tensor("out", [SEQ, DIM], F32, kind="ExternalOutput").ap()

    blob_np = make_const_blob()
    mm = nc.tensor.matmul

    with tile.TileContext(nc) as tc:
        with (
            tc.tile_pool(name="consts", bufs=1) as cpool,
            tc.tile_pool(name="xin", bufs=2) as xpool,
            tc.tile_pool(name="work", bufs=2) as spool,
            tc.tile_pool(name="psT", bufs=1, space="PSUM") as psT,
            tc.tile_pool(name="psP", bufs=2, space="PSUM") as psP,
            tc.tile_pool(name="psPT", bufs=1, space="PSUM") as psPT,
            tc.tile_pool(name="psW", bufs=2, space="PSUM") as psW,
            tc.tile_pool(name="psU", bufs=1, space="PSUM") as psU,
        ):
            # --- constants to SBUF. All DMAs serialize through a single
            # HWDGE (~630ns each), so the preamble issues in first-use order:
            # wk, xk0 (K projection = first PE work), wq, xq0, const blob,
            # wv, xv0. The weights DMA is split wk/wq/wv for this. ---
            w_sb = cpool.tile([128, 2, 3 * DIM], BF16, name="w_sb")
            wr = wqkv.rearrange("(h p) d -> p h d", p=128)
            blob_sb = cpool.tile([128, 900], BF16, name="blob_sb")
            idn_sb = blob_sb[:, 260:388]
            wv_sb = w_sb[:, :, 512:768]
            u_ps = psU.tile([128, 512], F32, name="u_state")

            def load_consts_first():
                nc.scalar.dma_start(out=w_sb[:, :, 256:512],
                                    in_=wr[:, :, 256:512])

            def load_consts_rest():
                nc.scalar.dma_start(out=w_sb[:, :, 0:256], in_=wr[:, :, 0:256])
                nc.sync.dma_start(out=blob_sb,
                                  in_=nc.inline_tensor(blob_np, "cblob").ap())
                nc.scalar.dma_start(out=w_sb[:, :, 512:768],
                                    in_=wr[:, :, 512:768])

            # persistent combined mask stationaries (manual quad-buffer for
            # the 2-superchunk lookahead): cols 0:125 = mpc (cross, rewritten
            # fully each superchunk); cols 125:375 = mpi region (shift trick:
            # write 130:255, main read 125:250, tail read 250:375); zeros
            # memset once. One fused op writes both segments (stride 130).
            mp_bufs = []
            for i_ in range(4):
                mb_ = spool.tile([125, 375], BF16, name=f"mp_{i_}", tag=f"mp_{i_}",
                                 bufs=1)
                nc.vector.memset(mb_[:, 125:130], 0.0)
                nc.vector.memset(mb_[:, 255:375], 0.0)
                mp_bufs.append(mb_)

            prev_mp = prev_v = None
            xg = {}
            qkt_sb = {}
            wpair = {}

            def load_x(gidx):
                gsl = slice(gidx * LGP, (gidx + 1) * LGP)
                xq_g = xpool.tile([128, 2, LGP], BF16, name=f"xq_{gidx}", tag="xq")
                xk_g = xpool.tile([128, 2, LGP], BF16, name=f"xk_{gidx}", tag="xk")
                xv_g = xpool.tile([128, 2, LGP], BF16, name=f"xv_{gidx}", tag="xv")
                # K first (first consumer); group 0's xk rides the ACT queue
                # so it issues in parallel with SP's xq
                eng_q = nc.scalar if gidx == 0 else nc.sync
                eng_q.dma_start(out=xk_g, in_=xkT[:, gsl].rearrange("(h p) a -> p h a", p=128))
                nc.sync.dma_start(out=xq_g, in_=xqT[:, gsl].rearrange("(h p) a -> p h a", p=128))
                nc.sync.dma_start(out=xv_g, in_=xvT[:, gsl].rearrange("(h p) a -> p h a", p=128))
                xg[gidx] = (xq_g, xk_g, xv_g)

            def project_group(gidx):
                """Q~^T / K~^T projections for the group (N=500), via a shared
                psum tag: d-lo cols 0:500 (bank 0), d-hi cols 512:1012
                (bank 1), each bank one closed accumulation group. K first so
                its evac (needed by the boundary transpose + P~^T) overlaps
                the Q projection."""
                xq_g, xk_g, xv_g = xg[gidx]
                qt_sb = spool.tile([128, 1000], BF16, name=f"qt_{gidx}", tag="qt")
                kt_sb = spool.tile([128, 1000], BF16, name=f"kt_{gidx}", tag="kt")
                qkt_k = psT.tile([128, 1024], F32, name=f"qkt_k_{gidx}", tag="qkt")
                for off, dlo in ((0, 256), (512, 384)):
                    for h in (0, 1):
                        mm(qkt_k[:, off:off + LGP], w_sb[:, h, dlo:dlo + 128],
                           xk_g[:, h, :], start=(h == 0), stop=(h == 1))
                nc.scalar.activation(
                    kt_sb.rearrange("p (a b) -> p a b", b=500),
                    qkt_k.rearrange("p (a b) -> p a b", b=512)[:, :, 0:500], COPY)
                qkt_q = psT.tile([128, 1024], F32, name=f"qkt_q_{gidx}", tag="qkt")
                for off, dlo in ((0, 0), (512, 128)):
                    for h in (0, 1):
                        mm(qkt_q[:, off:off + LGP], w_sb[:, h, dlo:dlo + 128],
                           xq_g[:, h, :], start=(h == 0), stop=(h == 1))
                nc.scalar.activation(
                    qt_sb.rearrange("p (a b) -> p a b", b=500),
                    qkt_q.rearrange("p (a b) -> p a b", b=512)[:, :, 0:500], COPY)
                qkt_sb[gidx] = (qt_sb, kt_sb)

            def prep_sc(s):
                """K~/V + P~^T + masked stationaries for superchunk s.
                Group g+1's x-load fires at ls==1 and its projections at
                ls==2, two superchunks before the boundary, so the boundary
                never waits on the qt/kt evacuations."""
                gidx, ls = divmod(s, LG)
                if ls == 1 and gidx + 1 < NSC // LG:
                    load_x(gidx + 1)
                if ls == 2 and gidx + 1 < NSC // LG:
                    project_group(gidx + 1)
                    xg.pop(gidx - 1, None)
                    qkt_sb.pop(gidx - 1, None)
                xq_g, xk_g, xv_g = xg[gidx]
                qt_sb, kt_sb = qkt_sb[gidx]
                lsl = slice(ls * GP, (ls + 1) * GP)
                xv_s = xv_g[:, :, lsl]
                qlo = qt_sb[:, ls * GP:(ls + 1) * GP]
                qhi = qt_sb[:, 500 + ls * GP:500 + (ls + 1) * GP]
                klo = kt_sb[:, ls * GP:(ls + 1) * GP]
                khi = kt_sb[:, 500 + ls * GP:500 + (ls + 1) * GP]

                # K~ pos-major via PE transpose of K~^T (bf16, written into
                # the f32 kv bank's first 512B via bitcast views); V pos-major
                # projected from x (f32 cols 256:512)
                kv = psP.tile([125, 512], F32, name=f"kv_{s}", tag="kv")
                nc.tensor.transpose(kv[:, 0:64].bitcast(BF16), klo, idn_sb)
                nc.tensor.transpose(kv[:, 64:128].bitcast(BF16), khi, idn_sb)
                for h in (0, 1):
                    mm(kv[:, 256:512], xv_s[:, h, :], wv_sb[:, h, :],
                       start=(h == 0), stop=(h == 1))
                kv_sb = spool.tile([125, 512], BF16, name=f"kv_sb_{s}", tag="kvsb", bufs=4)
                nc.vector.tensor_copy(kv_sb[:, 0:256], kv[:, 0:128].bitcast(BF16))
                nc.vector.tensor_copy(kv_sb[:, 256:512], kv[:, 256:512])
                k_sb = kv_sb[:, 0:256]
                v_sb = kv_sb[:, 256:512]

                # P~^T = K~ @ Q~^T (bf16: 1 cycle/row at any N)
                pt_ps = psPT.tile([125, 125], F32, name=f"pt_{s}", tag="pt")
                mm(pt_ps, klo, qlo, start=True, stop=False)
                mm(pt_ps, khi, qhi, start=False, stop=True)
                pt_sb = spool.tile([125, 125], BF16, name=f"pt_sb_{s}", tag="ptsb",
                                   bufs=3)
                nc.vector.tensor_copy(pt_sb, pt_ps)

                # one fused GPSIMD op: {mpc (cols 0:125), shifted mpi
                # (130:255)} = P~^T (read twice, stride-0) * {wct, wit}
                mp_sb = mp_bufs[s % 4]
                mp_out = mp_sb[:, 0:260].rearrange(
                    "p (a b) -> p a b", b=130)[:, :, 0:125]
                masks_in = blob_sb[0:GP, C_WCT:C_WCT + 260].rearrange(
                    "p (a b) -> p a b", b=130)[:, :, 0:125]
                pt_in = pt_sb.unsqueeze(1).broadcast_to((GP, 2, 125))
                nc.gpsimd.tensor_mul(mp_out, pt_in, masks_in)
                return dict(k_sb=k_sb, v_sb=v_sb, mp_sb=mp_sb, qlo=qlo, qhi=qhi)

            load_consts_first()          # wk — gate of the first projection
            load_x(0)
            load_consts_rest()
            project_group(0)
            # zero-matmul initializes the U bank's data + has_written bits so
            # the per-superchunk state matmuls can all accumulate; emitted
            # after the projections so its const-blob wait can't stall them
            nc.tensor.matmul(u_ps, blob_sb[0:1, 388:516], blob_sb[0:1, 388:900],
                             start=True, stop=True, skip_group_check=True)
            sts = {0: prep_sc(0), 1: prep_sc(1)}
            for s in range(NSC):
                st = sts.pop(s)
                k_sb, v_sb = st["k_sb"], st["v_sb"]
                mp_sb = st["mp_sb"]
                qlo, qhi = st["qlo"], st["qhi"]

                # --- window accumulation (one closed group). The seam
                # (intra tail of chunk s*G-1) is added directly from the
                # previous superchunk's tail stationary and V: rows 5:125 of
                # that matmul multiply zero columns and accumulate zeros. ---
                wt = psW.tile([125, 256], F32, name=f"wt_{s}", tag="wt")
                mm(wt, mp_sb[:, 0:125], v_sb, start=True, stop=False)
                mm(wt, mp_sb[:, 125:250], v_sb, start=False, stop=(s == 0))
                if s > 0:
                    ut_sb = spool.tile([128, 512], BF16, name=f"ut_{s}", tag="ut")
                    nc.scalar.activation(ut_sb, u_ps, COPY)
                    mm(wt, prev_mp[:, 250:375], prev_v, start=False, stop=False)
                    mm(wt, qlo, ut_sb[:, 0:256], start=False, stop=False)
                    mm(wt, qhi, ut_sb[:, 256:512], start=False, stop=True)

                # --- state update (accumulates; U bank bits set by zero-mm) ---
                mm(u_ps[:, 0:256], k_sb[:, 0:128], v_sb,
                   start=False, stop=True, skip_group_check=True)
                mm(u_ps[:, 256:512], k_sb[:, 128:256], v_sb,
                   start=False, stop=True, skip_group_check=True)

                # pipeline: prepare s+2 (two superchunks of lookahead) so the
                # P~^T -> evac -> GPSIMD mask chain and the kv/qt/kt
                # evacuations are never on the PE's critical path
                if s + 2 < NSC:
                    sts[s + 2] = prep_sc(s + 2)

                # --- output: DVE evac; superchunks 1..30 pair up (two windows
                # per [125,512] tile, one DMA per pair) to halve DGE issues ---
                if s == 0:
                    wall_sb = spool.tile([125, 256], F32, name="wall_0",
                                         tag="wall0")
                    nc.vector.tensor_copy(wall_sb, wt)
                    nc.sync.dma_start(out=out[0:GP - B], in_=wall_sb[B:GP])
                elif s == NSC - 1:
                    wall_sb = spool.tile([125, 256], F32, name=f"wall_{s}",
                                         tag="wallz")
                    nc.vector.tensor_copy(wall_sb, wt)
                    nc.scalar.dma_start(out=out[s * GP - B: s * GP - B + GP],
                                        in_=wall_sb)
                else:
                    half = (s - 1) % 2
                    if half == 0:
                        wpair["t"] = spool.tile([125, 512], F32,
                                                name=f"wallp_{s}", tag="wallp",
                                                bufs=2)
                    wp = wpair["t"]
                    nc.vector.tensor_copy(wp[:, half * 256:half * 256 + 256], wt)
                    if half == 1:
                        base = (s - 1) * GP - B
                        nc.sync.dma_start(
                            out=out[base: base + 2 * GP].rearrange(
                                "(a p) d -> p a d", a=2),
                            in_=wp)
                prev_mp, prev_v = mp_sb, v_sb

            # final output chunk 799 = intra tail of the last superchunk
            wtf = psW.tile([125, 256], F32, name="wt_final", tag="wt")
            mm(wtf, prev_mp[:, 250:375], prev_v, start=True, stop=True)
            wallf_sb = spool.tile([5, 256], F32, name="wallf", tag="wallf")
            nc.vector.tensor_copy(wallf_sb, wtf[0:5])
            nc.sync.dma_start(out=out[SEQ - B:SEQ], in_=wallf_sb)

    return nc


def _col_scales():
    j = np.arange(SEQ) // B          # global chunk index
    sq = (np.float64(g6) ** j).astype(np.float32)
    sk = (np.float64(g6) ** (-j)).astype(np.float32)
    return sq, sk


def prep_core_inputs(xq2d, xk2d, xv2d, wqkv):
    sq, sk = _col_scales()
    bf = ml_dtypes.bfloat16
    return {
        "xqT": np.ascontiguousarray((xq2d.T * sq[None, :]).astype(bf)),
        "xkT": np.ascontiguousarray((xk2d.T * sk[None, :]).astype(bf)),
        "xvT": np.ascontiguousarray(xv2d.T.astype(bf)),
        "wqkv": wqkv,
    }


def make_in_maps(inputs):
    """inputs: dict from setup_inputs (full batch). Returns per-core in_maps."""
    xq, xk, xv = inputs["xq"], inputs["xk"], inputs["xv"]
    wqkv = np.ascontiguousarray(np.concatenate(
        [np.asarray(inputs["Wq"], dtype=np.float32),
         np.asarray(inputs["Wk"], dtype=np.float32),
         np.asarray(inputs["Wv"], dtype=np.float32)],
        axis=1).astype(ml_dtypes.bfloat16))
    in_maps = []
    for b in range(8):
        in_maps.append(prep_core_inputs(
            np.asarray(xq[b], dtype=np.float32),
            np.asarray(xk[b], dtype=np.float32),
            np.asarray(xv[b], dtype=np.float32), wqkv))
    return in_maps


_NC_CACHE = {}


def _get_nc():
    if "nc" not in _NC_CACHE:
        from concourse import bacc
        nc = bacc.Bacc("TRN2", target_bir_lowering=False, debug=False)
        build_kernel(nc)
        nc.compile()
        _NC_CACHE["nc"] = nc
    return _NC_CACHE["nc"]


def run(inputs, trace=False, **kwargs):
    """Run on 8 NeuronCores; returns (output [8,4000,256], BassKernelResults)."""
    from concourse.bass_utils import run_bass_kernel_spmd

    nc = _get_nc()
    in_maps = make_in_maps(inputs)
    res = run_bass_kernel_spmd(nc, in_maps, core_ids=list(range(8)),
                               trace=trace, **kwargs)
    out = np.stack([r["out"] for r in res.results], axis=0)
    return out, res


def kernel(**inputs) -> np.ndarray:
    out, _ = run(inputs)
    return out
